# revision 12
# baseline (speedup 1.0000x reference)
"""Trainium2 Bass kernel for nn_Correlation (FlowNet-style 1-D correlation).

out[b, d, h, w] = mean_c( left[b,c,h,w] * right[b,c,h,w+d-40] ), d in [0,81),
with right zero-padded along W.  Inputs left/right: [4, 256, 128, 416] fp32.

Strategy (per NeuronCore; the 512 (b,h) rows are sharded over 8 cores by H):
  * out[:, :, h, :] is the 81-wide band of the Gram matrix
    G[w, w'] = sum_c L[c, w] R[c, w'] (contraction C=256 = 2x128 partition
    halves accumulated in fp32 PSUM).  Each 128-column W-tile streams a
    ~208-column window of R through the PE (clipped at the W edges).
  * Inputs are cast to fp16 on the host (halves HBM traffic, 1 PE
    cycle/column); the output band is stored fp16 and upcast on the host
    (end-to-end error ~5e-4 absmax-relative, tolerance is 2e-2).
  * Band layout: compute engines cannot apply per-partition column
    shifts, so the 81 diagonals cannot be compacted on-chip.  Rows of
    each 128-wide W-tile are grouped by G=32: the PSUM->SBUF copy places
    group a's window (psum cols [G*a, G*a + G+80)) at a fixed Bt column
    block, so row p holds out[d, w0+p] at col (p mod G) + d -- only
    (G+80)/81 = 1.38x the band bytes.  The device stores Bt DENSELY
    (contiguous >=896B runs -- sub-512B HBM writes eat a ~15x
    read-modify-write/descriptor penalty, measured); the host extracts
    the diagonals with a zero-copy strided view (shape [ng, G, nh, D],
    strides [G*rp, rp+1, 336, 1]) and transposes to [B, D, H, W].
    (This replaces v3's DRAM bounce -- sheared write 2.8x + diagonal
    reload + PE transpose + store, 3.8x band bytes total.)
  * DMA issues alternate between the SP and ACT HWDGE rings per h-chunk
    so one chunk's sequencer-blocking waits don't stall the next chunk's
    issues (HWDGE waits execute on the issuing sequencer on TRN2).
  * split_dma_waits legalizes Tile's multi-wait instructions for walrus,
    whose NEURON_ISA_TPB_EVENTS descriptor block holds a single sem wait:
    extra waits are hoisted onto the issuing sequencer as one-wait no-ops.
"""

import sys

sys.path.insert(0, "/opt/trn_rl_repo")

from contextlib import ExitStack

import numpy as np

import concourse.bass as bass
import concourse.tile as tile
from concourse import mybir

B, C, H, W = 4, 256, 128, 416
MD = 40
D = 2 * MD + 1  # 81 displacement channels
NCORES = 8
HS = H // NCORES  # 16 H-rows per core

W0S = [0, 128, 256, 384]  # w-tile starts
MS = [128, 128, 128, 32]  # w-tile widths

NH = 4  # h-rows per input DMA / store batch
GROUP = 32  # shear group rows; band block width per w-tile = GROUP + 80
BUFS = {"inp": 4, "work": 4, "psg": 8}


def _geom(grp, nh):
    gj = [min(grp, m) for m in MS]
    wv = [g + 2 * MD for g in gj]  # Bt col block width per tile
    bt0 = [0]
    for j in range(3):
        bt0.append(bt0[-1] + wv[j])
    btw = bt0[-1] + wv[3]
    blk0 = 128 * nh * (bt0[3])  # dense dump: [128, nh, bt0[3]] ...
    spc = blk0 + MS[3] * nh * wv[3]  # ... + [32, nh, wv3] elems per chunk
    return gj, wv, bt0, btw, blk0, spc


def _windows():
    """Per-tile stream windows over UNPADDED right coords: (a_j, n_j)."""
    res = []
    for w0, m in zip(W0S, MS):
        r0 = w0 - MD
        lo = max(0, -r0)
        hi = min(m + 2 * MD, W - r0)
        res.append((r0 + lo, hi - lo))
    return res


def corr_kernel(tc, outs, ins, hs=HS, nh=NH, grp=GROUP, bufs=None, reps=1,
                rings=None):
    nc = tc.nc
    left, right = ins["left"], ins["right"]
    out = outs["out"]  # [B, (hs/nh) * spc] fp16 dense band blocks

    wins = _windows()
    in_dt = mybir.dt.float16
    psum_n = max(n for _, n in wins)
    bufs = bufs or {}
    GJ, WV, BT0, BTW, BLK0, SPC = _geom(grp, nh)
    OFF = [W0S[j] - MD - wins[j][0] for j in range(4)]  # psum col of r=w0-40

    rings = rings or {}
    alternate = rings.get("alternate", True)

    def eng(k, parity=0):
        base = {"L": "sync", "R": "sync", "S": "scalar"}
        name = rings.get(k, base[k])
        if alternate and parity % 2 == 1:
            name = {"sync": "scalar", "scalar": "sync"}[name]
        return getattr(nc, name)

    with ExitStack() as ctx:
        inp = ctx.enter_context(tc.tile_pool(name="inp", bufs=bufs.get("inp", 4)))
        work = ctx.enter_context(tc.tile_pool(name="work", bufs=bufs.get("work", 4)))
        psg = ctx.enter_context(
            tc.tile_pool(name="psg", bufs=bufs.get("psg", 8), space="PSUM")
        )

        # W-edge zero regions of Bt (psum cols outside the valid window) are
        # identical every chunk and never overwritten by the copies: zero
        # them ONCE per pool buffer instead of re-memsetting per chunk.
        zregs = []
        for j in range(4):
            gj, wv, b0, off = GJ[j], WV[j], BT0[j], OFF[j]
            n = wins[j][1]
            for ai in range(MS[j] // gj):
                base = off + gj * ai
                c_lo = max(0, -base)
                c_hi = min(wv, n - base)
                if c_lo > 0:
                    zregs.append((gj * ai, gj * ai + gj, b0, b0 + c_lo))
                if c_hi < wv:
                    zregs.append((gj * ai, gj * ai + gj, b0 + c_hi, b0 + wv))
        for _ in range(bufs.get("work", 4)):
            Btz = work.tile([128, nh, BTW], in_dt, tag="B")
            for r0, r1, c0, c1 in zregs:
                nc.vector.memset(Btz[r0:r1, :, c0:c1], 0.0)

        def one_chunk(b, hc, par):
            L4 = inp.tile([128, 2, nh * W], in_dt, tag="L")
            eng("L", par).dma_start(
                L4[:],
                left[b, :, hc * nh : (hc + 1) * nh, :].rearrange(
                    "(t p) h w -> p t (h w)", p=128
                ),
            )
            R4 = inp.tile([128, 2, nh * W], in_dt, tag="R")
            eng("R", par).dma_start(
                R4[:],
                right[b, :, hc * nh : (hc + 1) * nh, :].rearrange(
                    "(t p) h w -> p t (h w)", p=128
                ),
            )

            # PSUM tiles hold an hl-PAIR of windows (2*208*4B = 1664B <= one
            # 2KB bank) so each PSUM->SBUF copy moves two h-rows: halves the
            # op count (per-op overhead, not bytes, bounds this stage).  All
            # ops of one PSUM tile stay on ONE engine -- DVE and ACT can only
            # access PSUM concurrently on different banks, so a per-op split
            # would serialize anyway; alternating per tile runs them truly in
            # parallel on different banks.
            Bt = work.tile([128, nh, BTW], in_dt, tag="B")
            for hp in range(nh // 2):
                for j in range(4):
                    a, n = wins[j]
                    g = psg.tile([128, 2, psum_n], mybir.dt.float32, tag="g")
                    for u in range(2):
                        hw0 = (2 * hp + u) * W
                        for t in range(2):
                            lhsT = L4[:, t, hw0 + W0S[j] : hw0 + W0S[j] + MS[j]]
                            rhs = R4[:, t, hw0 + a : hw0 + a + n]
                            nc.tensor.matmul(
                                g[0 : MS[j], u, 0:n], lhsT, rhs,
                                start=(t == 0), stop=(t == 1),
                            )
                    on_dve = (hp * 4 + j) % 2 == 0
                    gj, wv, b0, off = GJ[j], WV[j], BT0[j], OFF[j]
                    h0, h1 = 2 * hp, 2 * hp + 2
                    for ai in range(MS[j] // gj):
                        r0, r1 = gj * ai, gj * ai + gj
                        base = off + gj * ai  # psum col of Bt col 0
                        c_lo = max(0, -base)
                        c_hi = min(wv, n - base)
                        src = g[r0:r1, :, base + c_lo : base + c_hi]
                        dst = Bt[r0:r1, h0:h1, b0 + c_lo : b0 + c_hi]
                        if on_dve:
                            nc.vector.tensor_scalar_mul(dst, src, 1.0 / C)
                        else:
                            nc.scalar.mul(dst, src, 1.0 / C)

            # dense dump of the band blocks (host gathers the diagonals):
            # block0 = Bt[0:128, :, 0:BT0[3]] (runs nh*BT0[3]*2 B),
            # block1 = Bt[0:32, :, BT0[3]:BTW] (j3 rows only).
            ch0 = hc * SPC
            dst = out[b, ch0:]
            dd = dst.ap
            dd.clear()
            dd.extend([[nh * BT0[3], 128], [BT0[3], nh], [1, BT0[3]]])
            dst.ap = dd
            eng("S", par).dma_start(dst, Bt[:, :, 0 : BT0[3]])
            dst = out[b, ch0 + BLK0 :]
            dd = dst.ap
            dd.clear()
            dd.extend([[nh * WV[3], MS[3]], [WV[3], nh], [1, WV[3]]])
            dst.ap = dd
            eng("S", par).dma_start(dst, Bt[0 : MS[3], :, BT0[3] : BTW])

        assert hs % nh == 0
        ci = 0
        for _rep in range(reps):
            for b in range(B):
                for hc in range(hs // nh):
                    one_chunk(b, hc, ci)
                    ci += 1


def split_dma_waits(nc):
    """Legalize for walrus: instruction descriptors hold ONE sync wait
    (NEURON_ISA_TPB_EVENTS), but Tile attaches up to ~3.  Move the extras to
    standalone InstEventSemaphore waits on the instruction's engine right
    before it -- sequencers execute (and enqueue HWDGE descriptors) in
    program order, so the hoisted waits still guard the instruction."""
    n = 0
    for fn in nc.m.functions:
        for bb in fn.blocks:
            insts = bb.instructions
            out = []
            for inst in insts:
                si = getattr(inst, "sync_info", None)
                eng = getattr(inst, "engine", None)
                if (
                    si is not None
                    and si.on_wait
                    and len(si.on_wait) > 1
                    and eng is not None
                    and eng != mybir.EngineType.Unassigned
                ):
                    waits = list(si.on_wait)
                    for w in waits[:-1]:
                        ev = mybir.InstNoOp(name=f"{inst.name}-prewait{n}")
                        ev.engine = eng
                        ev.sync_info = mybir.SyncInfo(on_wait=[w], on_update=[])
                        nc.register_instruction(ev)
                        out.append(ev)
                        n += 1
                    inst.sync_info = mybir.SyncInfo(
                        on_wait=waits[-1:], on_update=list(si.on_update or [])
                    )
                out.append(inst)
            bb.instructions = out
    return n


def build_nc(hs=HS, nh=NH, grp=GROUP, reps=1, bufs=None):
    in_dt = mybir.dt.float16
    nc = bass.Bass(
        trn_type="TRN2", target_bir_lowering=False, debug=False, num_devices=NCORES
    )
    _, _, _, _, _, spc = _geom(grp, nh)
    ins = {
        "left": nc.dram_tensor("left", [B, C, hs, W], in_dt, kind="ExternalInput").ap(),
        "right": nc.dram_tensor(
            "right", [B, C, hs, W], in_dt, kind="ExternalInput"
        ).ap(),
    }
    outs = {
        "out": nc.dram_tensor(
            "out", [B, (hs // nh) * spc], in_dt, kind="ExternalOutput"
        ).ap()
    }
    with tile.TileContext(nc) as tc:
        corr_kernel(
            tc, outs, ins, hs=hs, nh=nh, grp=grp, bufs=(bufs or BUFS), reps=reps
        )
    split_dma_waits(nc)
    return nc


def make_in_maps(left, right):
    in_maps = []
    for i in range(NCORES):
        sl = slice(i * HS, (i + 1) * HS)
        in_maps.append(
            {
                "left": np.ascontiguousarray(left[:, :, sl, :]).astype(np.float16),
                "right": np.ascontiguousarray(right[:, :, sl, :]).astype(np.float16),
            }
        )
    return in_maps


def extract_out(flat, hs=HS, nh=NH, grp=GROUP):
    """[B, (hs/nh)*spc] fp16 dense band blocks -> [B, D, hs, W] fp32.

    Row p of tile j holds out[d, W0S[j]+p] at block col (p mod G) + d.
    """
    gj, wv, bt0, btw, blk0, spc = _geom(grp, nh)
    nch = hs // nh
    a = np.ascontiguousarray(np.asarray(flat)).reshape(B, nch, spc)
    res = np.empty((B, D, hs, W), np.float32)
    st = a.strides  # bytes
    for j in range(4):
        g = gj[j]
        ng = MS[j] // g
        rowp = nh * (bt0[3] if j < 3 else wv[3])  # elems per p row
        roww = bt0[3] if j < 3 else wv[3]  # elems per hl step
        off = bt0[j] if j < 3 else blk0
        # elem offset in chunk = off + (g*aa+q)*rowp + hl*roww + q + d
        v = np.lib.stride_tricks.as_strided(
            a[:, :, off:],
            shape=(B, nch, ng, g, nh, D),
            strides=(st[0], st[1], g * rowp * 2, (rowp + 1) * 2, roww * 2, 2),
        )
        blk = v.transpose(0, 5, 1, 4, 2, 3).reshape(B, D, hs, MS[j])
        res[:, :, :, W0S[j] : W0S[j] + MS[j]] = blk
    return res


def kernel(left, right):
    """Full-input entry point: [4,256,128,416] fp32 x2 -> [4,81,128,416] fp32."""
    from concourse.bass_utils import run_bass_kernel_spmd

    left = np.asarray(left, dtype=np.float32)
    right = np.asarray(right, dtype=np.float32)
    nc = build_nc()
    in_maps = make_in_maps(left, right)
    res = run_bass_kernel_spmd(nc, in_maps, list(range(NCORES)))
    return np.concatenate(
        [extract_out(res.results[i]["out"]) for i in range(NCORES)], axis=2
    )


if __name__ == "__main__":
    rng = np.random.default_rng(0)
    lf = rng.standard_normal((B, C, H, W), dtype=np.float32)
    rt = rng.standard_normal((B, C, H, W), dtype=np.float32)
    o = kernel(left=lf, right=rt)
    print(o.shape, o.dtype)


# revision 13
# speedup vs baseline: 1.1891x; 1.1891x over previous
"""Trainium2 Bass kernel for nn_Correlation (FlowNet-style 1-D correlation).

out[b, d, h, w] = mean_c( left[b,c,h,w] * right[b,c,h,w+d-40] ), d in [0,81),
with right zero-padded along W.  Inputs left/right: [4, 256, 128, 416] fp32.

Strategy (per NeuronCore; the 512 (b,h) rows are sharded over 8 cores by H):
  * out[:, :, h, :] is the 81-wide band of the Gram matrix
    G[w, w'] = sum_c L[c, w] R[c, w'] (contraction C=256 = 2x128 partition
    halves accumulated in fp32 PSUM).  Each 128-column W-tile streams a
    ~208-column window of R through the PE (clipped at the W edges).
  * Inputs are cast to fp16 on the host (halves HBM traffic, 1 PE
    cycle/column); the output band is stored fp16 and upcast on the host
    (end-to-end error ~5e-4 absmax-relative, tolerance is 2e-2).
  * Band layout: compute engines cannot apply per-partition column
    shifts, so the 81 diagonals cannot be compacted on-chip.  Rows of
    each 128-wide W-tile are grouped by G=32: the PSUM->SBUF copy places
    group a's window (psum cols [G*a, G*a + G+80)) at a fixed Bt column
    block, so row p holds out[d, w0+p] at col (p mod G) + d -- only
    (G+80)/81 = 1.38x the band bytes.  The device stores Bt DENSELY
    (contiguous >=896B runs -- sub-512B HBM writes eat a ~15x
    read-modify-write/descriptor penalty, measured); the host extracts
    the diagonals with a zero-copy strided view (shape [ng, G, nh, D],
    strides [G*rp, rp+1, 336, 1]) and transposes to [B, D, H, W].
    (This replaces v3's DRAM bounce -- sheared write 2.8x + diagonal
    reload + PE transpose + store, 3.8x band bytes total.)
  * DMA issues alternate between the SP and ACT HWDGE rings per h-chunk
    so one chunk's sequencer-blocking waits don't stall the next chunk's
    issues (HWDGE waits execute on the issuing sequencer on TRN2).
  * split_dma_waits legalizes Tile's multi-wait instructions for walrus,
    whose NEURON_ISA_TPB_EVENTS descriptor block holds a single sem wait:
    extra waits are hoisted onto the issuing sequencer as one-wait no-ops.
"""

import sys

sys.path.insert(0, "/opt/trn_rl_repo")

from contextlib import ExitStack

import numpy as np

import concourse.bass as bass
import concourse.tile as tile
from concourse import mybir

B, C, H, W = 4, 256, 128, 416
MD = 40
D = 2 * MD + 1  # 81 displacement channels
NCORES = 8
HS = H // NCORES  # 16 H-rows per core

W0S = [0, 128, 256, 384]  # w-tile starts
MS = [128, 128, 128, 32]  # w-tile widths

NH = 8  # h-rows per input DMA / store batch
GROUP = 64  # shear group rows; band block width per w-tile = GROUP + 80
BUFS = {"inp": 3, "work": 4, "psg": 8}


def _geom(grp, nh):
    gj = [min(grp, m) for m in MS]
    wv = [g + 2 * MD for g in gj]  # Bt col block width per tile
    bt0 = [0]
    for j in range(3):
        bt0.append(bt0[-1] + wv[j])
    btw = bt0[-1] + wv[3]
    blk0 = 128 * nh * (bt0[3])  # dense dump: [128, nh, bt0[3]] ...
    spc = blk0 + MS[3] * nh * wv[3]  # ... + [32, nh, wv3] elems per chunk
    return gj, wv, bt0, btw, blk0, spc


def _windows():
    """Per-tile stream windows over UNPADDED right coords: (a_j, n_j)."""
    res = []
    for w0, m in zip(W0S, MS):
        r0 = w0 - MD
        lo = max(0, -r0)
        hi = min(m + 2 * MD, W - r0)
        res.append((r0 + lo, hi - lo))
    return res


def corr_kernel(tc, outs, ins, hs=HS, nh=NH, grp=GROUP, bufs=None, reps=1,
                rings=None):
    nc = tc.nc
    left, right = ins["left"], ins["right"]
    out = outs["out"]  # [B, (hs/nh) * spc] fp16 dense band blocks

    wins = _windows()
    in_dt = mybir.dt.float16
    psum_n = max(n for _, n in wins)
    bufs = bufs or {}
    GJ, WV, BT0, BTW, BLK0, SPC = _geom(grp, nh)
    OFF = [W0S[j] - MD - wins[j][0] for j in range(4)]  # psum col of r=w0-40

    rings = rings or {}
    alternate = rings.get("alternate", True)

    def eng(k, parity=0):
        base = {"L": "sync", "R": "sync", "S": "scalar"}
        name = rings.get(k, base[k])
        if alternate and parity % 2 == 1:
            name = {"sync": "scalar", "scalar": "sync"}[name]
        return getattr(nc, name)

    with ExitStack() as ctx:
        inp = ctx.enter_context(tc.tile_pool(name="inp", bufs=bufs.get("inp", 4)))
        work = ctx.enter_context(tc.tile_pool(name="work", bufs=bufs.get("work", 4)))
        psg = ctx.enter_context(
            tc.tile_pool(name="psg", bufs=bufs.get("psg", 8), space="PSUM")
        )

        # W-edge zero regions of Bt (psum cols outside the valid window) are
        # identical every chunk and never overwritten by the copies: zero
        # them ONCE per pool buffer instead of re-memsetting per chunk.
        zregs = []
        for j in range(4):
            gj, wv, b0, off = GJ[j], WV[j], BT0[j], OFF[j]
            n = wins[j][1]
            for ai in range(MS[j] // gj):
                base = off + gj * ai
                c_lo = max(0, -base)
                c_hi = min(wv, n - base)
                if c_lo > 0:
                    zregs.append((gj * ai, gj * ai + gj, b0, b0 + c_lo))
                if c_hi < wv:
                    zregs.append((gj * ai, gj * ai + gj, b0 + c_hi, b0 + wv))
        for _ in range(bufs.get("work", 4)):
            Btz = work.tile([128, nh, BTW], in_dt, tag="B")
            for r0, r1, c0, c1 in zregs:
                nc.vector.memset(Btz[r0:r1, :, c0:c1], 0.0)

        def one_chunk(b, hc, par):
            L4 = inp.tile([128, 2, nh * W], in_dt, tag="L")
            eng("L", par).dma_start(
                L4[:],
                left[b, :, hc * nh : (hc + 1) * nh, :].rearrange(
                    "(t p) h w -> p t (h w)", p=128
                ),
            )
            R4 = inp.tile([128, 2, nh * W], in_dt, tag="R")
            eng("R", par).dma_start(
                R4[:],
                right[b, :, hc * nh : (hc + 1) * nh, :].rearrange(
                    "(t p) h w -> p t (h w)", p=128
                ),
            )

            # PSUM tiles hold an hl-PAIR of windows (2*208*4B = 1664B <= one
            # 2KB bank) so each PSUM->SBUF copy moves two h-rows: halves the
            # op count (per-op overhead, not bytes, bounds this stage).  All
            # ops of one PSUM tile stay on ONE engine -- DVE and ACT can only
            # access PSUM concurrently on different banks, so a per-op split
            # would serialize anyway; alternating per tile runs them truly in
            # parallel on different banks.
            Bt = work.tile([128, nh, BTW], in_dt, tag="B")
            for hp in range(nh // 2):
                for j in range(4):
                    a, n = wins[j]
                    g = psg.tile([128, 2, psum_n], mybir.dt.float32, tag="g")
                    for u in range(2):
                        hw0 = (2 * hp + u) * W
                        for t in range(2):
                            lhsT = L4[:, t, hw0 + W0S[j] : hw0 + W0S[j] + MS[j]]
                            rhs = R4[:, t, hw0 + a : hw0 + a + n]
                            nc.tensor.matmul(
                                g[0 : MS[j], u, 0:n], lhsT, rhs,
                                start=(t == 0), stop=(t == 1),
                            )
                    on_dve = (hp * 4 + j) % 2 == 0
                    gj, wv, b0, off = GJ[j], WV[j], BT0[j], OFF[j]
                    h0, h1 = 2 * hp, 2 * hp + 2
                    for ai in range(MS[j] // gj):
                        r0, r1 = gj * ai, gj * ai + gj
                        base = off + gj * ai  # psum col of Bt col 0
                        c_lo = max(0, -base)
                        c_hi = min(wv, n - base)
                        src = g[r0:r1, :, base + c_lo : base + c_hi]
                        dst = Bt[r0:r1, h0:h1, b0 + c_lo : b0 + c_hi]
                        if on_dve:
                            nc.vector.tensor_scalar_mul(dst, src, 1.0 / C)
                        else:
                            nc.scalar.mul(dst, src, 1.0 / C)

            # dense dump of the band blocks (host gathers the diagonals):
            # block0 = Bt[0:128, :, 0:BT0[3]] (runs nh*BT0[3]*2 B),
            # block1 = Bt[0:32, :, BT0[3]:BTW] (j3 rows only).
            ch0 = hc * SPC
            dst = out[b, ch0:]
            dd = dst.ap
            dd.clear()
            dd.extend([[nh * BT0[3], 128], [BT0[3], nh], [1, BT0[3]]])
            dst.ap = dd
            eng("S", par).dma_start(dst, Bt[:, :, 0 : BT0[3]])
            dst = out[b, ch0 + BLK0 :]
            dd = dst.ap
            dd.clear()
            dd.extend([[nh * WV[3], MS[3]], [WV[3], nh], [1, WV[3]]])
            dst.ap = dd
            eng("S", par).dma_start(dst, Bt[0 : MS[3], :, BT0[3] : BTW])

        assert hs % nh == 0
        ci = 0
        for _rep in range(reps):
            for b in range(B):
                for hc in range(hs // nh):
                    one_chunk(b, hc, ci)
                    ci += 1


def split_dma_waits(nc):
    """Legalize for walrus: instruction descriptors hold ONE sync wait
    (NEURON_ISA_TPB_EVENTS), but Tile attaches up to ~3.  Move the extras to
    standalone InstEventSemaphore waits on the instruction's engine right
    before it -- sequencers execute (and enqueue HWDGE descriptors) in
    program order, so the hoisted waits still guard the instruction."""
    n = 0
    for fn in nc.m.functions:
        for bb in fn.blocks:
            insts = bb.instructions
            out = []
            for inst in insts:
                si = getattr(inst, "sync_info", None)
                eng = getattr(inst, "engine", None)
                if (
                    si is not None
                    and si.on_wait
                    and len(si.on_wait) > 1
                    and eng is not None
                    and eng != mybir.EngineType.Unassigned
                ):
                    waits = list(si.on_wait)
                    for w in waits[:-1]:
                        ev = mybir.InstNoOp(name=f"{inst.name}-prewait{n}")
                        ev.engine = eng
                        ev.sync_info = mybir.SyncInfo(on_wait=[w], on_update=[])
                        nc.register_instruction(ev)
                        out.append(ev)
                        n += 1
                    inst.sync_info = mybir.SyncInfo(
                        on_wait=waits[-1:], on_update=list(si.on_update or [])
                    )
                out.append(inst)
            bb.instructions = out
    return n


def build_nc(hs=HS, nh=NH, grp=GROUP, reps=1, bufs=None):
    in_dt = mybir.dt.float16
    nc = bass.Bass(
        trn_type="TRN2", target_bir_lowering=False, debug=False, num_devices=NCORES
    )
    _, _, _, _, _, spc = _geom(grp, nh)
    ins = {
        "left": nc.dram_tensor("left", [B, C, hs, W], in_dt, kind="ExternalInput").ap(),
        "right": nc.dram_tensor(
            "right", [B, C, hs, W], in_dt, kind="ExternalInput"
        ).ap(),
    }
    outs = {
        "out": nc.dram_tensor(
            "out", [B, (hs // nh) * spc], in_dt, kind="ExternalOutput"
        ).ap()
    }
    with tile.TileContext(nc) as tc:
        corr_kernel(
            tc, outs, ins, hs=hs, nh=nh, grp=grp, bufs=(bufs or BUFS), reps=reps
        )
    split_dma_waits(nc)
    return nc


def make_in_maps(left, right):
    in_maps = []
    for i in range(NCORES):
        sl = slice(i * HS, (i + 1) * HS)
        in_maps.append(
            {
                "left": np.ascontiguousarray(left[:, :, sl, :]).astype(np.float16),
                "right": np.ascontiguousarray(right[:, :, sl, :]).astype(np.float16),
            }
        )
    return in_maps


def extract_out(flat, hs=HS, nh=NH, grp=GROUP):
    """[B, (hs/nh)*spc] fp16 dense band blocks -> [B, D, hs, W] fp32.

    Row p of tile j holds out[d, W0S[j]+p] at block col (p mod G) + d.
    """
    gj, wv, bt0, btw, blk0, spc = _geom(grp, nh)
    nch = hs // nh
    a = np.ascontiguousarray(np.asarray(flat)).reshape(B, nch, spc)
    res = np.empty((B, D, hs, W), np.float32)
    st = a.strides  # bytes
    for j in range(4):
        g = gj[j]
        ng = MS[j] // g
        rowp = nh * (bt0[3] if j < 3 else wv[3])  # elems per p row
        roww = bt0[3] if j < 3 else wv[3]  # elems per hl step
        off = bt0[j] if j < 3 else blk0
        # elem offset in chunk = off + (g*aa+q)*rowp + hl*roww + q + d
        v = np.lib.stride_tricks.as_strided(
            a[:, :, off:],
            shape=(B, nch, ng, g, nh, D),
            strides=(st[0], st[1], g * rowp * 2, (rowp + 1) * 2, roww * 2, 2),
        )
        blk = v.transpose(0, 5, 1, 4, 2, 3).reshape(B, D, hs, MS[j])
        res[:, :, :, W0S[j] : W0S[j] + MS[j]] = blk
    return res


def kernel(left, right):
    """Full-input entry point: [4,256,128,416] fp32 x2 -> [4,81,128,416] fp32."""
    from concourse.bass_utils import run_bass_kernel_spmd

    left = np.asarray(left, dtype=np.float32)
    right = np.asarray(right, dtype=np.float32)
    nc = build_nc()
    in_maps = make_in_maps(left, right)
    res = run_bass_kernel_spmd(nc, in_maps, list(range(NCORES)))
    return np.concatenate(
        [extract_out(res.results[i]["out"]) for i in range(NCORES)], axis=2
    )


if __name__ == "__main__":
    rng = np.random.default_rng(0)
    lf = rng.standard_normal((B, C, H, W), dtype=np.float32)
    rt = rng.standard_normal((B, C, H, W), dtype=np.float32)
    o = kernel(left=lf, right=rt)
    print(o.shape, o.dtype)


# revision 14
# speedup vs baseline: 1.2363x; 1.0397x over previous
"""Trainium2 Bass kernel for nn_Correlation (FlowNet-style 1-D correlation).

out[b, d, h, w] = mean_c( left[b,c,h,w] * right[b,c,h,w+d-40] ), d in [0,81),
with right zero-padded along W.  Inputs left/right: [4, 256, 128, 416] fp32.

Strategy (per NeuronCore; the 512 (b,h) rows are sharded over 8 cores by H):
  * out[:, :, h, :] is the 81-wide band of the Gram matrix
    G[w, w'] = sum_c L[c, w] R[c, w'] (contraction C=256 = 2x128 partition
    halves accumulated in fp32 PSUM).  Each 128-column W-tile streams a
    ~208-column window of R through the PE (clipped at the W edges).
  * Inputs are cast to fp16 on the host (halves HBM traffic, 1 PE
    cycle/column); the output band is stored fp16 and upcast on the host
    (end-to-end error ~5e-4 absmax-relative, tolerance is 2e-2).
  * Band layout: compute engines cannot apply per-partition column
    shifts, so the 81 diagonals cannot be compacted on-chip.  Rows of
    each 128-wide W-tile are grouped by G=32: the PSUM->SBUF copy places
    group a's window (psum cols [G*a, G*a + G+80)) at a fixed Bt column
    block, so row p holds out[d, w0+p] at col (p mod G) + d -- only
    (G+80)/81 = 1.38x the band bytes.  The device stores Bt DENSELY
    (contiguous >=896B runs -- sub-512B HBM writes eat a ~15x
    read-modify-write/descriptor penalty, measured); the host extracts
    the diagonals with a zero-copy strided view (shape [ng, G, nh, D],
    strides [G*rp, rp+1, 336, 1]) and transposes to [B, D, H, W].
    (This replaces v3's DRAM bounce -- sheared write 2.8x + diagonal
    reload + PE transpose + store, 3.8x band bytes total.)
  * DMA issues alternate between the SP and ACT HWDGE rings per h-chunk
    so one chunk's sequencer-blocking waits don't stall the next chunk's
    issues (HWDGE waits execute on the issuing sequencer on TRN2).
  * split_dma_waits legalizes Tile's multi-wait instructions for walrus,
    whose NEURON_ISA_TPB_EVENTS descriptor block holds a single sem wait:
    extra waits are hoisted onto the issuing sequencer as one-wait no-ops.
"""

import sys

sys.path.insert(0, "/opt/trn_rl_repo")

from contextlib import ExitStack

import numpy as np

import concourse.bass as bass
import concourse.tile as tile
from concourse import mybir

B, C, H, W = 4, 256, 128, 416
MD = 40
D = 2 * MD + 1  # 81 displacement channels
NCORES = 8
HS = H // NCORES  # 16 H-rows per core

W0S = [0, 128, 256, 384]  # w-tile starts
MS = [128, 128, 128, 32]  # w-tile widths

NH = 4  # h-rows per input DMA / store batch
GROUP = 64  # shear group rows; band block width per w-tile = GROUP + 80
BUFS = {"inp": 4, "work": 4, "psg": 8}


def _geom(grp, nh):
    gj = [min(grp, m) for m in MS]
    wv = [g + 2 * MD for g in gj]  # Bt col block width per tile
    bt0 = [0]
    for j in range(3):
        bt0.append(bt0[-1] + wv[j])
    btw = bt0[-1] + wv[3]
    blk0 = 128 * nh * (bt0[3])  # dense dump: [128, nh, bt0[3]] ...
    spc = blk0 + MS[3] * nh * wv[3]  # ... + [32, nh, wv3] elems per chunk
    return gj, wv, bt0, btw, blk0, spc


def _windows():
    """Per-tile stream windows over UNPADDED right coords: (a_j, n_j)."""
    res = []
    for w0, m in zip(W0S, MS):
        r0 = w0 - MD
        lo = max(0, -r0)
        hi = min(m + 2 * MD, W - r0)
        res.append((r0 + lo, hi - lo))
    return res


def corr_kernel(tc, outs, ins, hs=HS, nh=NH, grp=GROUP, bufs=None, reps=1,
                rings=None):
    nc = tc.nc
    left, right = ins["left"], ins["right"]
    out = outs["out"]  # [B, (hs/nh) * spc] fp16 dense band blocks

    wins = _windows()
    in_dt = mybir.dt.float16
    psum_n = max(n for _, n in wins)
    bufs = bufs or {}
    GJ, WV, BT0, BTW, BLK0, SPC = _geom(grp, nh)
    OFF = [W0S[j] - MD - wins[j][0] for j in range(4)]  # psum col of r=w0-40

    rings = rings or {}
    alternate = rings.get("alternate", True)

    def eng(k, parity=0):
        base = {"L": "sync", "R": "sync", "S": "scalar"}
        name = rings.get(k, base[k])
        if alternate and parity % 2 == 1:
            name = {"sync": "scalar", "scalar": "sync"}[name]
        return getattr(nc, name)

    with ExitStack() as ctx:
        inp = ctx.enter_context(tc.tile_pool(name="inp", bufs=bufs.get("inp", 4)))
        work = ctx.enter_context(tc.tile_pool(name="work", bufs=bufs.get("work", 4)))
        psg = ctx.enter_context(
            tc.tile_pool(name="psg", bufs=bufs.get("psg", 8), space="PSUM")
        )

        # W-edge zero regions of Bt (psum cols outside the valid window) are
        # identical every chunk and never overwritten by the copies: zero
        # them ONCE per pool buffer instead of re-memsetting per chunk.
        zregs = []
        for j in range(4):
            gj, wv, b0, off = GJ[j], WV[j], BT0[j], OFF[j]
            n = wins[j][1]
            for ai in range(MS[j] // gj):
                base = off + gj * ai
                c_lo = max(0, -base)
                c_hi = min(wv, n - base)
                if c_lo > 0:
                    zregs.append((gj * ai, gj * ai + gj, b0, b0 + c_lo))
                if c_hi < wv:
                    zregs.append((gj * ai, gj * ai + gj, b0 + c_hi, b0 + wv))
        for _ in range(bufs.get("work", 4)):
            Btz = work.tile([128, nh, BTW], in_dt, tag="B")
            for r0, r1, c0, c1 in zregs:
                nc.vector.memset(Btz[r0:r1, :, c0:c1], 0.0)

        def one_chunk(b, hc, par):
            L4 = inp.tile([128, 2, nh * W], in_dt, tag="L")
            eng("L", par).dma_start(
                L4[:],
                left[b, :, hc * nh : (hc + 1) * nh, :].rearrange(
                    "(t p) h w -> p t (h w)", p=128
                ),
            )
            R4 = inp.tile([128, 2, nh * W], in_dt, tag="R")
            eng("R", par).dma_start(
                R4[:],
                right[b, :, hc * nh : (hc + 1) * nh, :].rearrange(
                    "(t p) h w -> p t (h w)", p=128
                ),
            )

            # PSUM tiles hold an hl-PAIR of windows (2*208*4B = 1664B <= one
            # 2KB bank) so each PSUM->SBUF copy moves two h-rows: halves the
            # op count (per-op overhead, not bytes, bounds this stage).  All
            # ops of one PSUM tile stay on ONE engine -- DVE and ACT can only
            # access PSUM concurrently on different banks, so a per-op split
            # would serialize anyway; alternating per tile runs them truly in
            # parallel on different banks.
            Bt = work.tile([128, nh, BTW], in_dt, tag="B")
            for hp in range(nh // 2):
                for j in range(4):
                    a, n = wins[j]
                    g = psg.tile([128, 2, psum_n], mybir.dt.float32, tag="g")
                    for u in range(2):
                        hw0 = (2 * hp + u) * W
                        for t in range(2):
                            lhsT = L4[:, t, hw0 + W0S[j] : hw0 + W0S[j] + MS[j]]
                            rhs = R4[:, t, hw0 + a : hw0 + a + n]
                            nc.tensor.matmul(
                                g[0 : MS[j], u, 0:n], lhsT, rhs,
                                start=(t == 0), stop=(t == 1),
                            )
                    on_dve = (hp * 4 + j) % 2 == 0
                    gj, wv, b0, off = GJ[j], WV[j], BT0[j], OFF[j]
                    h0, h1 = 2 * hp, 2 * hp + 2
                    for ai in range(MS[j] // gj):
                        r0, r1 = gj * ai, gj * ai + gj
                        base = off + gj * ai  # psum col of Bt col 0
                        c_lo = max(0, -base)
                        c_hi = min(wv, n - base)
                        src = g[r0:r1, :, base + c_lo : base + c_hi]
                        dst = Bt[r0:r1, h0:h1, b0 + c_lo : b0 + c_hi]
                        if on_dve:
                            nc.vector.tensor_scalar_mul(dst, src, 1.0 / C)
                        else:
                            nc.scalar.mul(dst, src, 1.0 / C)

            # dense dump of the band blocks (host gathers the diagonals):
            # block0 = Bt[0:128, :, 0:BT0[3]] (runs nh*BT0[3]*2 B),
            # block1 = Bt[0:32, :, BT0[3]:BTW] (j3 rows only).
            ch0 = hc * SPC
            dst = out[b, ch0:]
            dd = dst.ap
            dd.clear()
            dd.extend([[nh * BT0[3], 128], [BT0[3], nh], [1, BT0[3]]])
            dst.ap = dd
            eng("S", par).dma_start(dst, Bt[:, :, 0 : BT0[3]])
            dst = out[b, ch0 + BLK0 :]
            dd = dst.ap
            dd.clear()
            dd.extend([[nh * WV[3], MS[3]], [WV[3], nh], [1, WV[3]]])
            dst.ap = dd
            eng("S", par).dma_start(dst, Bt[0 : MS[3], :, BT0[3] : BTW])

        assert hs % nh == 0
        ci = 0
        for _rep in range(reps):
            for b in range(B):
                for hc in range(hs // nh):
                    one_chunk(b, hc, ci)
                    ci += 1


def split_dma_waits(nc):
    """Legalize for walrus: instruction descriptors hold ONE sync wait
    (NEURON_ISA_TPB_EVENTS), but Tile attaches up to ~3.  Move the extras to
    standalone InstEventSemaphore waits on the instruction's engine right
    before it -- sequencers execute (and enqueue HWDGE descriptors) in
    program order, so the hoisted waits still guard the instruction."""
    n = 0
    for fn in nc.m.functions:
        for bb in fn.blocks:
            insts = bb.instructions
            out = []
            for inst in insts:
                si = getattr(inst, "sync_info", None)
                eng = getattr(inst, "engine", None)
                if (
                    si is not None
                    and si.on_wait
                    and len(si.on_wait) > 1
                    and eng is not None
                    and eng != mybir.EngineType.Unassigned
                ):
                    waits = list(si.on_wait)
                    for w in waits[:-1]:
                        ev = mybir.InstNoOp(name=f"{inst.name}-prewait{n}")
                        ev.engine = eng
                        ev.sync_info = mybir.SyncInfo(on_wait=[w], on_update=[])
                        nc.register_instruction(ev)
                        out.append(ev)
                        n += 1
                    inst.sync_info = mybir.SyncInfo(
                        on_wait=waits[-1:], on_update=list(si.on_update or [])
                    )
                out.append(inst)
            bb.instructions = out
    return n


def build_nc(hs=HS, nh=NH, grp=GROUP, reps=1, bufs=None):
    in_dt = mybir.dt.float16
    nc = bass.Bass(
        trn_type="TRN2", target_bir_lowering=False, debug=False, num_devices=NCORES
    )
    _, _, _, _, _, spc = _geom(grp, nh)
    ins = {
        "left": nc.dram_tensor("left", [B, C, hs, W], in_dt, kind="ExternalInput").ap(),
        "right": nc.dram_tensor(
            "right", [B, C, hs, W], in_dt, kind="ExternalInput"
        ).ap(),
    }
    outs = {
        "out": nc.dram_tensor(
            "out", [B, (hs // nh) * spc], in_dt, kind="ExternalOutput"
        ).ap()
    }
    with tile.TileContext(nc) as tc:
        corr_kernel(
            tc, outs, ins, hs=hs, nh=nh, grp=grp, bufs=(bufs or BUFS), reps=reps
        )
    split_dma_waits(nc)
    return nc


def make_in_maps(left, right):
    in_maps = []
    for i in range(NCORES):
        sl = slice(i * HS, (i + 1) * HS)
        in_maps.append(
            {
                "left": np.ascontiguousarray(left[:, :, sl, :]).astype(np.float16),
                "right": np.ascontiguousarray(right[:, :, sl, :]).astype(np.float16),
            }
        )
    return in_maps


def extract_out(flat, hs=HS, nh=NH, grp=GROUP):
    """[B, (hs/nh)*spc] fp16 dense band blocks -> [B, D, hs, W] fp32.

    Row p of tile j holds out[d, W0S[j]+p] at block col (p mod G) + d.
    """
    gj, wv, bt0, btw, blk0, spc = _geom(grp, nh)
    nch = hs // nh
    a = np.ascontiguousarray(np.asarray(flat)).reshape(B, nch, spc)
    res = np.empty((B, D, hs, W), np.float32)
    st = a.strides  # bytes
    for j in range(4):
        g = gj[j]
        ng = MS[j] // g
        rowp = nh * (bt0[3] if j < 3 else wv[3])  # elems per p row
        roww = bt0[3] if j < 3 else wv[3]  # elems per hl step
        off = bt0[j] if j < 3 else blk0
        # elem offset in chunk = off + (g*aa+q)*rowp + hl*roww + q + d
        v = np.lib.stride_tricks.as_strided(
            a[:, :, off:],
            shape=(B, nch, ng, g, nh, D),
            strides=(st[0], st[1], g * rowp * 2, (rowp + 1) * 2, roww * 2, 2),
        )
        blk = v.transpose(0, 5, 1, 4, 2, 3).reshape(B, D, hs, MS[j])
        res[:, :, :, W0S[j] : W0S[j] + MS[j]] = blk
    return res


def kernel(left, right):
    """Full-input entry point: [4,256,128,416] fp32 x2 -> [4,81,128,416] fp32."""
    from concourse.bass_utils import run_bass_kernel_spmd

    left = np.asarray(left, dtype=np.float32)
    right = np.asarray(right, dtype=np.float32)
    nc = build_nc()
    in_maps = make_in_maps(left, right)
    res = run_bass_kernel_spmd(nc, in_maps, list(range(NCORES)))
    return np.concatenate(
        [extract_out(res.results[i]["out"]) for i in range(NCORES)], axis=2
    )


if __name__ == "__main__":
    rng = np.random.default_rng(0)
    lf = rng.standard_normal((B, C, H, W), dtype=np.float32)
    rt = rng.standard_normal((B, C, H, W), dtype=np.float32)
    o = kernel(left=lf, right=rt)
    print(o.shape, o.dtype)
